# revision 1
# baseline (speedup 1.0000x reference)
"""Trainium2 Bass kernel for a 12-layer GPT LM (CodeGPTLMHeadModel).

Sharding (8 NeuronCores, one chip):
  - Layer stack: tokens sharded. B=2 batches x 1024 tokens; cores 0-3 own
    batch 0, cores 4-7 batch 1; core c owns 256 contiguous tokens
    (chunks 2a, 2a+1 with a = c%4).  All weights replicated, streamed
    from HBM.  Attention: each core computes q/k/v for its local tokens,
    AllGathers K^T/V inside its 4-core batch group, then computes all 16
    heads for its local 256 queries (causal handled by a per-core mask
    input => uniform SPMD graph).
  - LM head: vocab sharded.  AllGather of the final hidden states across
    all 8 cores; each core computes a 6656-wide padded vocab slice.
  - Norm weights (ln1/ln2/lnf) are folded into the following matmul
    weights host-side; qn/kn/gate are applied on-device from replicated
    constant inputs.  Matmuls run in bf16 (f32 residual/psum).
"""

import numpy as np
import ml_dtypes

BF16 = ml_dtypes.bfloat16

L_ALL, B, T, D, H, HD, F, V = 12, 2, 1024, 1024, 16, 64, 4096, 50257
NCORE = 8
TLOC = 256            # tokens per core
QT = TLOC // 128      # 2 token tiles of 128
NKT = D // 128        # 8 contraction tiles over D
NFT = F // 128        # 32 tiles over F
VS = 6656             # padded vocab shard per core (13 * 512)
NVC = VS // 512       # 13 vocab chunks of 512
EPS = 1e-5
LO_ROWS = 25088       # embed split for int16 gather indices (max idx 25088)
HI_ROWS = V - LO_ROWS  # 25169

# kv bounce layout (bf16): one row per partition: k^T 2048 cols + v 2*1040 cols
KV_COLS = 2048 + 2 * 1040  # 4128


def build_nc(n_layers=L_ALL):
    from contextlib import ExitStack
    from concourse import bass, bacc, mybir, tile

    f32 = mybir.dt.float32
    bf = mybir.dt.bfloat16
    i16 = mybir.dt.int16
    AF = mybir.ActivationFunctionType

    nc = bacc.Bacc(None, target_bir_lowering=False, debug=False)

    # ---------------- external parameters (per-core shards) ----------------
    x0_d = nc.dram_tensor("x0", [TLOC, D], f32, kind="ExternalInput")
    maskt = nc.dram_tensor("maskt", [8, 128, TLOC], bf, kind="ExternalInput")
    wq_d = nc.dram_tensor("wq", [n_layers, D, D], bf, kind="ExternalInput")
    wk_d = nc.dram_tensor("wk", [n_layers, D, D], bf, kind="ExternalInput")
    wv_d = nc.dram_tensor("wv", [n_layers, D, D], bf, kind="ExternalInput")
    wo_d = nc.dram_tensor("wo", [n_layers, D, D], bf, kind="ExternalInput")
    w1_d = nc.dram_tensor("w1", [n_layers, D, F], bf, kind="ExternalInput")
    w2_d = nc.dram_tensor("w2", [n_layers, F, D], bf, kind="ExternalInput")
    qnk_d = nc.dram_tensor("qnk", [n_layers, 2, 128, D], bf, kind="ExternalInput")
    g_d = nc.dram_tensor("g", [n_layers, 128, H], f32, kind="ExternalInput")
    wlm_d = nc.dram_tensor("wlm", [D, VS], bf, kind="ExternalInput")
    out_d = nc.dram_tensor("out", [B * T, VS], bf, kind="ExternalOutput")

    id_np = np.eye(128, dtype=BF16)
    id_dram = nc.inline_tensor(id_np, name="id128")

    with tile.TileContext(nc) as tc, ExitStack() as ctx:
        ep = ctx.enter_context

        consts = ep(tc.tile_pool(name="consts", bufs=1))
        p_res = ep(tc.tile_pool(name="p_res", bufs=1))
        p_h = ep(tc.tile_pool(name="p_h", bufs=3))
        p_tr = ep(tc.tile_pool(name="p_tr", bufs=1))
        p_small = ep(tc.tile_pool(name="p_small", bufs=8))
        p_v = ep(tc.tile_pool(name="p_v", bufs=1))
        p_o2 = ep(tc.tile_pool(name="p_o2", bufs=1))
        p_uT = ep(tc.tile_pool(name="p_uT", bufs=1))
        p_slab = ep(tc.tile_pool(name="p_slab", bufs=1))
        p_w = ep(tc.tile_pool(name="p_w", bufs=3))
        p_lmh = ep(tc.tile_pool(name="p_lmh", bufs=1))
        p_out = ep(tc.tile_pool(name="p_out", bufs=2))
        p_qn = ep(tc.tile_pool(name="p_qn", bufs=2))
        p_p = ep(tc.tile_pool(name="p_p", bufs=3))
        ps = ep(tc.tile_pool(name="ps", bufs=8, space="PSUM"))
        dram = ep(tc.tile_pool(name="dram", bufs=2, space="DRAM"))

        # ---------------- constants into SBUF ----------------
        id_sb = consts.tile([128, 128], bf, tag="id", name="id")
        nc.sync.dma_start(id_sb[:], id_dram[:, :])
        mask_sb = consts.tile([128, 8, TLOC], bf, tag="mask", name="mask")
        nc.sync.dma_start(mask_sb[:], maskt[:, :, :].rearrange("t p q -> p t q"))
        eps_sb = consts.tile([128, 1], f32, tag="eps", name="eps")
        nc.vector.memset(eps_sb[:], EPS)

        # ---------------- embedding ----------------
        x = [p_res.tile([128, D], f32, tag=f"x{q}", name=f"x{q}") for q in range(QT)]

        for q in range(QT):
            nc.sync.dma_start(x[q][:], x0_d[q * 128:(q + 1) * 128, :])

        # ---------------- helpers ----------------
        def rms_to_hT(tag):
            """RMS-normalize x (token-major) -> h bf16 -> transposed hT[128,8,256]."""
            hT = p_tr.tile([128, NKT, TLOC], bf, tag=tag)
            for q in range(QT):
                sq = p_h.tile([128, D], bf, tag="sc", name="sq")
                ssq = p_small.tile([128, 1], f32, tag="ssq", name="ssq")
                nc.scalar.activation(sq[:], x[q][:], AF.Square, accum_out=ssq[:])
                std = p_small.tile([128, 1], f32, tag="std", name="std")
                nc.scalar.activation(std[:], ssq[:], AF.Sqrt, scale=1.0 / D, bias=eps_sb[:])
                inv = p_small.tile([128, 1], f32, tag="inv", name="inv")
                nc.vector.reciprocal(inv[:], std[:])
                h = p_h.tile([128, D], bf, tag="h", name="h")
                nc.vector.tensor_scalar_mul(h[:], x[q][:], inv[:])
                for d in range(NKT):
                    pt = ps.tile([128, 512], bf, tag="ps", name="ps")
                    nc.tensor.transpose(pt[:, :128], h[:, d * 128:(d + 1) * 128], id_sb[:])
                    nc.vector.tensor_copy(hT[:, d, q * 128:(q + 1) * 128], pt[:, :128])
            return hT

        def proj_qkv(hT, w_dram, l):
            """form-1 projection: out[128tok, D] psum pair per (q, ch)."""
            outs = {}
            for ch in range(2):
                pts = [ps.tile([128, 512], f32, tag="ps", name="ps") for _ in range(QT)]
                wt = p_w.tile([128, NKT, 512], bf, tag="w", name="w")
                nc.sync.dma_start(
                    wt[:], w_dram[l, :, ch * 512:(ch + 1) * 512]
                    .rearrange("(a p) c -> p a c", p=128)
                )
                for k in range(NKT):
                    for q in range(QT):
                        nc.tensor.matmul(
                            pts[q][:], hT[:, k, q * 128:(q + 1) * 128], wt[:, k, :],
                            start=(k == 0), stop=(k == NKT - 1),
                        )
                outs[ch] = pts
            return outs

        def qknorm_transpose(pq, qn_sb, which, tag):
            """QK-norm (token-major, from psum) + qn/kn apply + transpose.

            pq: dict ch -> [QT] psum tiles [128, 512] (= [128, 8, 64])
            returns qT [128, 8, 256] bf16 (partition = dim%128 within pairs of heads)
            """
            qT = p_tr.tile([128, NKT, TLOC], bf, tag=tag)
            for q in range(QT):
                ss = p_small.tile([128, H], f32, tag="ssqk", name="ssqk")
                for ch in range(2):
                    sqs = p_h.tile([128, 512], bf, tag="sc", name="sqs")
                    nc.scalar.activation(sqs[:], pq[ch][q][:], AF.Square)
                    nc.vector.tensor_reduce(
                        ss[:, ch * 8:(ch + 1) * 8],
                        sqs[:].rearrange("p (h d) -> p h d", d=HD),
                        axis=mybir.AxisListType.X, op=mybir.AluOpType.add,
                    )
                st = p_small.tile([128, H], f32, tag="stqk", name="stqk")
                nc.scalar.activation(st[:], ss[:], AF.Sqrt, scale=1.0 / HD, bias=eps_sb[:])
                iv = p_small.tile([128, H], f32, tag="ivqk", name="ivqk")
                nc.vector.reciprocal(iv[:], st[:])
                qh = p_h.tile([128, D], bf, tag="qh", name="qh")
                for ch in range(2):
                    tmp = p_h.tile([128, 512], f32, tag="sc", name="qtmp")
                    nc.vector.tensor_tensor(
                        tmp[:].rearrange("p (h d) -> p h d", d=HD),
                        pq[ch][q][:].rearrange("p (h d) -> p h d", d=HD),
                        iv[:, ch * 8:(ch + 1) * 8, None].to_broadcast((128, 8, HD)),
                        op=mybir.AluOpType.mult,
                    )
                    nc.vector.tensor_mul(
                        qh[:, ch * 512:(ch + 1) * 512], tmp[:],
                        qn_sb[:, which, ch * 512:(ch + 1) * 512],
                    )
                for d in range(NKT):
                    pt = ps.tile([128, 512], bf, tag="ps", name="ps")
                    nc.tensor.transpose(pt[:, :128], qh[:, d * 128:(d + 1) * 128], id_sb[:])
                    nc.vector.tensor_copy(qT[:, d, q * 128:(q + 1) * 128], pt[:, :128])
            return qT

        # ---------------- layers ----------------
        for l in range(n_layers):
            qn_sb = p_qn.tile([128, 2, D], bf, tag="qn", name="qn")
            nc.sync.dma_start(qn_sb[:], qnk_d[l, :, :, :].rearrange("a p d -> p a d"))
            g_sb = p_qn.tile([128, H], f32, tag="g", name="g")
            nc.sync.dma_start(g_sb[:], g_d[l, :, :])

            hT = rms_to_hT("hT")

            # --- QKV projections ---
            pq = proj_qkv(hT, wq_d, l)
            pk = proj_qkv(hT, wk_d, l)
            pv = proj_qkv(hT, wv_d, l)

            # v eviction: [128, 16, 65] with ones in col 64
            v_sb = []
            for q in range(QT):
                vt = p_v.tile([128, H, HD + 1], bf, tag=f"v{q}", name=f"v{q}")
                for ch in range(2):
                    nc.scalar.activation(
                        vt[:, ch * 8:(ch + 1) * 8, :HD],
                        pv[ch][q][:].rearrange("p (h d) -> p h d", d=HD),
                        AF.Copy,
                    )
                nc.vector.memset(vt[:, :, HD:], 1.0)
                v_sb.append(vt)

            qT = qknorm_transpose(pq, qn_sb, 0, "qT")
            kT = qknorm_transpose(pk, qn_sb, 1, "kT")

            # --- bounce K^T and V to DRAM, AllGather within batch group ---
            kv_in = dram.tile([128, KV_COLS], bf, tag="kv_in", name="kv_in")
            kv_out = dram.tile([4 * 128, KV_COLS], bf, tag="kv_out", name="kv_out")
            nc.sync.dma_start(kv_in[:, 0:2048], kT[:])
            for q in range(QT):
                nc.sync.dma_start(
                    kv_in[:, 2048 + q * 1040: 2048 + (q + 1) * 1040], v_sb[q][:]
                )
            nc.gpsimd.collective_compute(
                "AllGather", mybir.AluOpType.bypass,
                ins=[kv_in[:].opt()], outs=[kv_out[:].opt()],
                replica_groups=[[0, 1, 2, 3], [4, 5, 6, 7]],
            )
            kTf = p_slab.tile([128, 4 * NKT, TLOC], bf, tag="kTf", name="kTf")
            vf = p_slab.tile([128, 8, H, HD + 1], bf, tag="vf", name="vf")
            for s in range(4):
                nc.sync.dma_start(
                    kTf[:, s * 8:(s + 1) * 8, :],
                    kv_out[s * 128:(s + 1) * 128, 0:2048],
                )
                nc.sync.dma_start(
                    vf[:, 2 * s:2 * s + 2, :, :],
                    kv_out[s * 128:(s + 1) * 128, 2048:KV_COLS],
                )

            # --- attention: all 16 heads, 8 key tiles, local 256 queries ---
            o2 = [p_o2.tile([128, H, HD], f32, tag=f"o2{q}", name=f"o2{q}") for q in range(QT)]
            for h in range(H):
                po = ps.tile([HD + 1, 512], f32, tag="ps", name="ps")
                qr = qT[64 * (h % 2): 64 * (h % 2) + 64, h // 2, :]
                for t in range(8):
                    pss = ps.tile([128, 512], f32, tag="ps", name="ps")
                    lk = kTf[64 * (h % 2): 64 * (h % 2) + 64,
                             (t // 2) * 8 + h // 2,
                             (t % 2) * 128: (t % 2) * 128 + 128]
                    nc.tensor.matmul(pss[:, :TLOC], lk, qr, start=True, stop=True)
                    pe = p_p.tile([128, TLOC], bf, tag="pe", name="pe")
                    nc.scalar.activation(pe[:], pss[:, :TLOC], AF.Exp)
                    nc.vector.tensor_mul(pe[:], pe[:], mask_sb[:, t, :])
                    nc.tensor.matmul(
                        po[:, :TLOC], vf[:, t, h, :], pe[:],
                        start=(t == 0), stop=(t == 7),
                    )
                # epilogue: transpose, divide by sums, gate
                ot = p_p.tile([HD + 1, TLOC], bf, tag="ot", name="ot")
                nc.scalar.activation(ot[:], po[:, :TLOC], AF.Copy)
                for q in range(QT):
                    px = ps.tile([128, 512], bf, tag="ps", name="ps")
                    nc.tensor.transpose(
                        px[:, :HD + 1], ot[:, q * 128:(q + 1) * 128],
                        id_sb[:HD + 1, :HD + 1],
                    )
                    inv = p_small.tile([128, 1], f32, tag="ainv", name="ainv")
                    nc.vector.reciprocal(inv[:], px[:, HD:HD + 1])
                    ivg = p_small.tile([128, 1], f32, tag="aivg", name="aivg")
                    nc.vector.tensor_mul(ivg[:], inv[:], g_sb[:, h:h + 1])
                    nc.scalar.activation(
                        o2[q][:, h, :], px[:, :HD], AF.Copy, scale=ivg[:]
                    )

            # value residual + transpose for Wo
            o2T = p_tr.tile([128, NKT, TLOC], bf, tag="o2T", name="o2T")
            for q in range(QT):
                nc.vector.tensor_add(o2[q][:], o2[q][:], v_sb[q][:, :, :HD])
                o2b = p_h.tile([128, D], bf, tag="sc", name="o2b")
                nc.vector.tensor_copy(o2b[:], o2[q][:].rearrange("p h d -> p (h d)"))
                for d in range(NKT):
                    pt = ps.tile([128, 512], bf, tag="ps", name="ps")
                    nc.tensor.transpose(pt[:, :128], o2b[:, d * 128:(d + 1) * 128], id_sb[:])
                    nc.vector.tensor_copy(o2T[:, d, q * 128:(q + 1) * 128], pt[:, :128])

            # --- Wo: x += o2 @ Wo ---
            for ch in range(2):
                pts = [ps.tile([128, 512], f32, tag="ps", name="ps") for _ in range(QT)]
                wt = p_w.tile([128, NKT, 512], bf, tag="w", name="w")
                nc.sync.dma_start(
                    wt[:], wo_d[l, :, ch * 512:(ch + 1) * 512]
                    .rearrange("(a p) c -> p a c", p=128)
                )
                for k in range(NKT):
                    for q in range(QT):
                        nc.tensor.matmul(
                            pts[q][:], o2T[:, k, q * 128:(q + 1) * 128], wt[:, k, :],
                            start=(k == 0), stop=(k == NKT - 1),
                        )
                for q in range(QT):
                    nc.vector.tensor_add(
                        x[q][:, ch * 512:(ch + 1) * 512],
                        x[q][:, ch * 512:(ch + 1) * 512], pts[q][:],
                    )

            # --- MLP ---
            hmT = rms_to_hT("hmT")
            uT = p_uT.tile([128, NFT, TLOC], bf, tag="uT", name="uT")
            for fg in range(NFT // 4):
                w1t = p_w.tile([128, NKT, 4, 128], bf, tag="w", name="w1")
                nc.sync.dma_start(
                    w1t[:],
                    w1_d[l, :, fg * 512:(fg + 1) * 512]
                    .rearrange("(a p) (b f) -> p a b f", p=128, f=128),
                )
                for fi in range(4):
                    fc = fg * 4 + fi
                    pu = ps.tile([128, 512], f32, tag="ps", name="ps")
                    for k in range(NKT):
                        nc.tensor.matmul(
                            pu[:, :TLOC], w1t[:, k, fi, :], hmT[:, k, :],
                            start=(k == 0), stop=(k == NKT - 1),
                        )
                    nc.scalar.activation(uT[:, fc, :], pu[:, :TLOC], AF.Gelu)
            for ch in range(2):
                pts = [ps.tile([128, 512], f32, tag="ps", name="ps") for _ in range(QT)]
                for fgg in range(4):
                    w2t = p_w.tile([128, NKT, 512], bf, tag="w", name="w")
                    nc.sync.dma_start(
                        w2t[:], w2_d[l, fgg * 1024:(fgg + 1) * 1024,
                                     ch * 512:(ch + 1) * 512]
                        .rearrange("(a p) c -> p a c", p=128)
                    )
                    for ki in range(NKT):
                        fc = fgg * 8 + ki
                        for q in range(QT):
                            nc.tensor.matmul(
                                pts[q][:], uT[:, fc, q * 128:(q + 1) * 128],
                                w2t[:, ki, :],
                                start=(fc == 0), stop=(fc == NFT - 1),
                            )
                for q in range(QT):
                    nc.vector.tensor_add(
                        x[q][:, ch * 512:(ch + 1) * 512],
                        x[q][:, ch * 512:(ch + 1) * 512], pts[q][:],
                    )

        # ---------------- final norm + LM head ----------------
        hfT = rms_to_hT("hT")
        lm_in = dram.tile([128, 2048], bf, tag="lm_in", name="lm_in")
        lm_out = dram.tile([8 * 128, 2048], bf, tag="lm_out", name="lm_out", addr_space="Shared")
        nc.sync.dma_start(lm_in[:, :], hfT[:])
        nc.gpsimd.collective_compute(
            "AllGather", mybir.AluOpType.bypass,
            ins=[lm_in[:].opt()], outs=[lm_out[:].opt()],
            replica_groups=[[0, 1, 2, 3, 4, 5, 6, 7]],
        )
        hfa = p_lmh.tile([128, 8 * NKT, TLOC], bf, tag="hfT", name="hfT")
        for s in range(8):
            nc.sync.dma_start(
                hfa[:, s * 8:(s + 1) * 8, :],
                lm_out[s * 128:(s + 1) * 128, :],
            )
        for vt in range(NVC):
            wl = p_w.tile([128, NKT, 512], bf, tag="w", name="wlm")
            nc.sync.dma_start(
                wl[:],
                wlm_d[:, vt * 512:(vt + 1) * 512]
                .rearrange("(a p) c -> p a c", p=128),
            )
            for tg in range(4):
                ob = p_out.tile([128, 4, 512], bf, tag="ob", name="ob")
                for ti in range(4):
                    tt = tg * 4 + ti
                    pl = ps.tile([128, 512], f32, tag="ps", name="ps")
                    for k in range(NKT):
                        nc.tensor.matmul(
                            pl[:],
                            hfa[:, (tt // 2) * 8 + k,
                                (tt % 2) * 128:(tt % 2) * 128 + 128],
                            wl[:, k, :],
                            start=(k == 0), stop=(k == NKT - 1),
                        )
                    nc.scalar.activation(ob[:, ti, :], pl[:], AF.Copy)
                nc.sync.dma_start(
                    out_d[tg * 512:(tg + 1) * 512, vt * 512:(vt + 1) * 512]
                    .rearrange("(b p) c -> p b c", p=128),
                    ob[:],
                )
    nc.compile()
    return nc


# ---------------------------------------------------------------------------
# host side
# ---------------------------------------------------------------------------

def _prep_inputs(inputs, n_layers=L_ALL):
    ids = np.asarray(inputs["input_ids"])
    embed = np.asarray(inputs["embed"], np.float32)
    pos = np.asarray(inputs["pos_embed"], np.float32)
    ln1 = np.asarray(inputs["ln1_w"], np.float32)
    ln2 = np.asarray(inputs["ln2_w"], np.float32)
    qn = np.asarray(inputs["qn_w"], np.float32)
    kn = np.asarray(inputs["kn_w"], np.float32)
    gate = np.asarray(inputs["gate"], np.float32)
    lnf = np.asarray(inputs["lnf_w"], np.float32)

    wq = (ln1[:, :, None] * np.asarray(inputs["Wq"], np.float32)).astype(BF16)
    wk = (ln1[:, :, None] * np.asarray(inputs["Wk"], np.float32)).astype(BF16)
    wv = (ln1[:, :, None] * np.asarray(inputs["Wv"], np.float32)).astype(BF16)
    wo = np.asarray(inputs["Wo"], np.float32).astype(BF16)
    w1 = (ln2[:, :, None] * np.asarray(inputs["W1"], np.float32)).astype(BF16)
    w2 = np.asarray(inputs["W2"], np.float32).astype(BF16)
    wlm_full = lnf[:, None] * np.asarray(inputs["Wlm"], np.float32)
    wlm_pad = np.zeros((D, VS * NCORE), np.float32)
    wlm_pad[:, :V] = wlm_full
    wlm_pad = wlm_pad.astype(BF16)

    # qn/kn replicated [L, 2, 128, D]; 1/sqrt(HD) folded into kn side
    qnk = np.zeros((n_layers, 2, 128, D), np.float32)
    for l in range(n_layers):
        qnk[l, 0, :, :] = np.tile(qn[l], H)[None, :]
        qnk[l, 1, :, :] = np.tile(kn[l] / np.sqrt(HD), H)[None, :]
    qnk = qnk.astype(BF16)
    g_rep = np.broadcast_to(gate[:n_layers, None, :], (n_layers, 128, H)).copy()

    # zero row at index 0; real rows shifted by +1
    in_maps = []
    for c in range(NCORE):
        b, a = divmod(c, 4)
        t0 = a * TLOC
        toks = ids[b, t0:t0 + TLOC].astype(np.int64)
        x0 = embed[toks] + pos[t0:t0 + TLOC]
        # causal mask [8 key tiles, 128 key pos, 256 query pos]
        kg = np.arange(T).reshape(8, 128)
        qg = t0 + np.arange(TLOC)
        mask = (kg[:, :, None] <= qg[None, None, :]).astype(BF16)
        in_maps.append({
            "x0": x0.astype(np.float32),
            "maskt": mask,
            "wq": wq[:n_layers], "wk": wk[:n_layers], "wv": wv[:n_layers],
            "wo": wo[:n_layers], "w1": w1[:n_layers], "w2": w2[:n_layers],
            "qnk": qnk, "g": g_rep.astype(np.float32),
            "wlm": wlm_pad[:, c * VS:(c + 1) * VS],
        })
    return in_maps


_NC_CACHE = {}


def _get_nc(n_layers=L_ALL):
    if n_layers not in _NC_CACHE:
        _NC_CACHE[n_layers] = build_nc(n_layers)
    return _NC_CACHE[n_layers]


def _install_profile_hook():
    """Recreate antenv.axon_hooks with an NTFF profile hook via ctypes."""
    import sys as _sys, types, ctypes, contextlib, os
    try:
        import antenv.axon_hooks  # noqa: F401
        return
    except ImportError:
        pass
    so_path = os.environ.get("PJRT_LIBRARY_PATH", "/opt/axon/libaxon_pjrt.so")
    lib = ctypes.CDLL(so_path)
    if not hasattr(lib, "axon_start_nrt_profile"):
        return
    lib.axon_start_nrt_profile.argtypes = [ctypes.POINTER(ctypes.c_int64), ctypes.c_size_t]
    lib.axon_start_nrt_profile.restype = ctypes.c_int64
    lib.axon_stop_nrt_profile.argtypes = [ctypes.c_char_p]
    lib.axon_stop_nrt_profile.restype = ctypes.c_int64

    @contextlib.contextmanager
    def _hook(output_dir, device_ids):
        import jax
        jax.devices()
        if device_ids:
            ids = (ctypes.c_int64 * len(device_ids))(*device_ids)
            rc = lib.axon_start_nrt_profile(ids, len(device_ids))
        else:
            rc = lib.axon_start_nrt_profile(None, 0)
        if rc != 0:
            raise RuntimeError(f"axon_start_nrt_profile rc={rc}")
        try:
            yield
        finally:
            n = lib.axon_stop_nrt_profile(str(output_dir).encode())
            print(f"profile: {n} file(s) written to {output_dir}")

    import antenv
    mod = types.ModuleType("antenv.axon_hooks")
    _state = {"hook": _hook}
    mod.set_axon_ntff_profile_hook = lambda h: _state.__setitem__("hook", h)
    mod.get_axon_ntff_profile_hook = lambda: _state["hook"]
    _sys.modules["antenv.axon_hooks"] = mod
    antenv.axon_hooks = mod


def run(inputs, n_layers=L_ALL, trace=False):
    from concourse.bass_utils import run_bass_kernel_spmd
    if trace:
        _install_profile_hook()
    nc = _get_nc(n_layers)
    in_maps = _prep_inputs(inputs, n_layers)
    res = run_bass_kernel_spmd(
        nc, in_maps, core_ids=list(range(NCORE)), trace=trace,
    )
    outs = [np.asarray(r["out"], dtype=np.float32) for r in res.results]
    logits = np.concatenate(outs, axis=1)[:, :V]
    return logits.reshape(B, T, V), res


def kernel(**inputs):
    logits, _ = run(inputs)
    return logits



# revision 5
# speedup vs baseline: 1.1050x; 1.1050x over previous
"""Trainium2 Bass kernel for a 12-layer GPT LM (CodeGPTLMHeadModel).

Sharding (8 NeuronCores, one chip):
  - Layer stack: tokens sharded. B=2 batches x 1024 tokens; cores 0-3 own
    batch 0, cores 4-7 batch 1; core c owns 256 contiguous tokens
    (chunks 2a, 2a+1 with a = c%4).  All weights replicated, streamed
    from HBM.  Attention: each core computes q/k/v for its local tokens,
    AllGathers K^T/V inside its 4-core batch group, then computes all 16
    heads for its local 256 queries (causal handled by a per-core mask
    input => uniform SPMD graph).
  - LM head: vocab sharded.  AllGather of the final hidden states across
    all 8 cores; each core computes a 6656-wide padded vocab slice.
  - Norm weights (ln1/ln2/lnf) are folded into the following matmul
    weights host-side; qn/kn/gate are applied on-device from replicated
    constant inputs.  Matmuls run in bf16 (f32 residual/psum).

v2 perf notes:
  - rsqrt via Ln+Exp so rms/qknorm/attention all live in the
    natural_log_exp activation table set (only Gelu switches sets).
  - attention: per head-pair, QK matmuls for 4 key tiles are issued
    interleaved (partitions 0-63 / 64-127 -> concurrent row-tiled MMs),
    exp runs on [128,1024] psum regions (1 ACT inst per 4 key tiles),
    mask is one tensor_tensor, AV matmuls run back-to-back.
  - K/V computed first, AllGather launched, all Q-side work overlaps it.
"""

import numpy as np
import ml_dtypes

BF16 = ml_dtypes.bfloat16

L_ALL, B, T, D, H, HD, F, V = 12, 2, 1024, 1024, 16, 64, 4096, 50257
NCORE = 8
TLOC = 256            # tokens per core
QT = TLOC // 128      # 2 token tiles of 128
NKT = D // 128        # 8 contraction tiles over D
NFT = F // 128        # 32 tiles over F
VS = 6656             # padded vocab shard per core (13 * 512)
NVC = VS // 512       # 13 vocab chunks of 512
EPS = 1e-5

# kv bounce layout (bf16): one row per partition: k^T 2048 cols + v 2*1040 cols
KV_COLS = 2048 + 2 * 1040  # 4128


def build_nc(n_layers=L_ALL):
    from contextlib import ExitStack
    from concourse import bass, bacc, mybir, tile

    f32 = mybir.dt.float32
    bf = mybir.dt.bfloat16
    AF = mybir.ActivationFunctionType

    nc = bacc.Bacc(None, target_bir_lowering=False, debug=False)

    # ---------------- external parameters (per-core shards) ----------------
    x0_d = nc.dram_tensor("x0", [TLOC, D], f32, kind="ExternalInput")
    maskt = nc.dram_tensor("maskt", [8, 128, TLOC], bf, kind="ExternalInput")
    wq_d = nc.dram_tensor("wq", [n_layers, D, D], bf, kind="ExternalInput")
    wk_d = nc.dram_tensor("wk", [n_layers, D, D], bf, kind="ExternalInput")
    wv_d = nc.dram_tensor("wv", [n_layers, D, D], bf, kind="ExternalInput")
    wo_d = nc.dram_tensor("wo", [n_layers, D, D], bf, kind="ExternalInput")
    w1_d = nc.dram_tensor("w1", [n_layers, D, F], bf, kind="ExternalInput")
    w2_d = nc.dram_tensor("w2", [n_layers, F, D], bf, kind="ExternalInput")
    qnk_d = nc.dram_tensor("qnk", [n_layers, 2, 128, D], bf, kind="ExternalInput")
    g_d = nc.dram_tensor("g", [n_layers, 128, H], f32, kind="ExternalInput")
    wlm_d = nc.dram_tensor("wlm", [D, VS], bf, kind="ExternalInput")
    out_d = nc.dram_tensor("out", [B * T, VS], bf, kind="ExternalOutput")

    id_np = np.eye(128, dtype=BF16)
    id_dram = nc.inline_tensor(id_np, name="id128")

    with tile.TileContext(nc) as tc, ExitStack() as ctx:
        ep = ctx.enter_context

        consts = ep(tc.tile_pool(name="consts", bufs=1))
        p_res = ep(tc.tile_pool(name="p_res", bufs=1))
        p_h = ep(tc.tile_pool(name="p_h", bufs=2))
        p_tr = ep(tc.tile_pool(name="p_tr", bufs=1))
        p_small = ep(tc.tile_pool(name="p_small", bufs=8))
        p_v = ep(tc.tile_pool(name="p_v", bufs=1))
        p_o2 = ep(tc.tile_pool(name="p_o2", bufs=1))
        p_uT = ep(tc.tile_pool(name="p_uT", bufs=1))
        p_slab = ep(tc.tile_pool(name="p_slab", bufs=1))
        p_w = ep(tc.tile_pool(name="p_w", bufs=4))
        p_lmh = ep(tc.tile_pool(name="p_lmh", bufs=1))
        p_out = ep(tc.tile_pool(name="p_out", bufs=2))
        p_qn = ep(tc.tile_pool(name="p_qn", bufs=2))
        p_pe = ep(tc.tile_pool(name="p_pe", bufs=4))
        # PSUM: 8 banks total.  ps_big = 2 x [128,1024] f32 (2 banks each),
        # ps = 4 x [128,512] f32 (1 bank each).
        ps_big = ep(tc.tile_pool(name="ps_big", bufs=2, space="PSUM"))
        ps = ep(tc.tile_pool(name="ps", bufs=4, space="PSUM"))
        dram = ep(tc.tile_pool(name="dram", bufs=2, space="DRAM"))

        # ---------------- constants into SBUF ----------------
        id_sb = consts.tile([128, 128], bf, tag="id", name="id")
        nc.sync.dma_start(id_sb[:], id_dram[:, :])
        mask_sb = consts.tile([128, 8, TLOC], bf, tag="mask", name="mask")
        nc.sync.dma_start(mask_sb[:], maskt[:, :, :].rearrange("t p q -> p t q"))
        eps_sb = consts.tile([128, 1], f32, tag="eps", name="eps")
        nc.vector.memset(eps_sb[:], EPS)

        # ---------------- embedding ----------------
        x = [p_res.tile([128, D], f32, tag=f"x{q}", name=f"x{q}") for q in range(QT)]

        for q in range(QT):
            nc.sync.dma_start(x[q][:], x0_d[q * 128:(q + 1) * 128, :])

        # ---------------- helpers ----------------
        def rsqrt_act(dst, src, scale):
            """dst = rsqrt(src*scale + eps) using Ln+Exp (stays in the
            natural_log_exp table set alongside attention's Exp)."""
            lt = p_small.tile(list(dst.shape), f32, tag="lnt", name="lnt")
            nc.scalar.activation(lt[:], src, AF.Ln, scale=scale, bias=eps_sb[:])
            nc.scalar.activation(dst, lt[:], AF.Exp, scale=-0.5)

        def rms_to_hT(tag):
            """RMS-normalize x (token-major) -> h bf16 -> transposed hT[128,8,256]."""
            hT = p_tr.tile([128, NKT, TLOC], bf, tag=tag)
            for q in range(QT):
                sq = p_h.tile([128, D], bf, tag="sc", name="sq")
                ssq = p_small.tile([128, 1], f32, tag="ssq", name="ssq")
                nc.scalar.activation(sq[:], x[q][:], AF.Square, accum_out=ssq[:])
                inv = p_small.tile([128, 1], f32, tag="inv", name="inv")
                rsqrt_act(inv[:], ssq[:], 1.0 / D)
                h = p_h.tile([128, D], bf, tag="h", name="h")
                nc.vector.tensor_scalar_mul(h[:], x[q][:], inv[:])
                for d in range(NKT):
                    pt = ps.tile([128, 512], bf, tag="ps", name="ps")
                    nc.tensor.transpose(pt[:, :128], h[:, d * 128:(d + 1) * 128], id_sb[:])
                    nc.vector.tensor_copy(hT[:, d, q * 128:(q + 1) * 128], pt[:, :128])
            return hT

        def proj_qkv(hT, w_dram, l):
            """form-1 projection: out[128tok, D] psum pair per (q, ch)."""
            outs = {}
            for ch in range(2):
                pts = [ps.tile([128, 512], f32, tag="ps", name="ps") for _ in range(QT)]
                wt = p_w.tile([128, NKT, 512], bf, tag="w", name="w")
                nc.sync.dma_start(
                    wt[:], w_dram[l, :, ch * 512:(ch + 1) * 512]
                    .rearrange("(a p) c -> p a c", p=128)
                )
                for k in range(NKT):
                    for q in range(QT):
                        nc.tensor.matmul(
                            pts[q][:], hT[:, k, q * 128:(q + 1) * 128], wt[:, k, :],
                            start=(k == 0), stop=(k == NKT - 1),
                        )
                outs[ch] = pts
            return outs

        def qknorm_transpose(pq, qn_sb, which, tag):
            """QK-norm (token-major, from psum) + qn/kn apply + transpose.

            pq: dict ch -> [QT] psum tiles [128, 512] (= [128, 8, 64])
            returns qT [128, 8, 256] bf16 (partition = dim%128 within pairs of heads)
            """
            qT = p_tr.tile([128, NKT, TLOC], bf, tag=tag)
            for q in range(QT):
                ss = p_small.tile([128, H], f32, tag="ssqk", name="ssqk")
                for ch in range(2):
                    sqs = p_h.tile([128, 512], bf, tag="sc", name="sqs")
                    nc.scalar.activation(sqs[:], pq[ch][q][:], AF.Square)
                    nc.vector.tensor_reduce(
                        ss[:, ch * 8:(ch + 1) * 8],
                        sqs[:].rearrange("p (h d) -> p h d", d=HD),
                        axis=mybir.AxisListType.X, op=mybir.AluOpType.add,
                    )
                iv = p_small.tile([128, H], f32, tag="ivqk", name="ivqk")
                rsqrt_act(iv[:], ss[:], 1.0 / HD)
                qh = p_h.tile([128, D], bf, tag="qh", name="qh")
                for ch in range(2):
                    tmp = p_h.tile([128, 512], f32, tag="sc", name="qtmp")
                    nc.vector.tensor_tensor(
                        tmp[:].rearrange("p (h d) -> p h d", d=HD),
                        pq[ch][q][:].rearrange("p (h d) -> p h d", d=HD),
                        iv[:, ch * 8:(ch + 1) * 8, None].to_broadcast((128, 8, HD)),
                        op=mybir.AluOpType.mult,
                    )
                    nc.vector.tensor_mul(
                        qh[:, ch * 512:(ch + 1) * 512], tmp[:],
                        qn_sb[:, which, ch * 512:(ch + 1) * 512],
                    )
                for d in range(NKT):
                    pt = ps.tile([128, 512], bf, tag="ps", name="ps")
                    nc.tensor.transpose(pt[:, :128], qh[:, d * 128:(d + 1) * 128], id_sb[:])
                    nc.vector.tensor_copy(qT[:, d, q * 128:(q + 1) * 128], pt[:, :128])
            return qT

        # ---------------- layers ----------------
        for l in range(n_layers):
            qn_sb = p_qn.tile([128, 2, D], bf, tag="qn", name="qn")
            nc.sync.dma_start(qn_sb[:], qnk_d[l, :, :, :].rearrange("a p d -> p a d"))
            g_sb = p_qn.tile([128, H], f32, tag="g", name="g")
            nc.sync.dma_start(g_sb[:], g_d[l, :, :])

            hT = rms_to_hT("hT")

            # --- K and V first, so the AllGather can launch early ---
            pk = proj_qkv(hT, wk_d, l)
            kT = qknorm_transpose(pk, qn_sb, 1, "kT")

            pv = proj_qkv(hT, wv_d, l)
            # v eviction: [128, 16, 65] with ones in col 64
            v_sb = []
            for q in range(QT):
                vt = p_v.tile([128, H, HD + 1], bf, tag=f"v{q}", name=f"v{q}")
                for ch in range(2):
                    nc.vector.tensor_copy(
                        vt[:, ch * 8:(ch + 1) * 8, :HD],
                        pv[ch][q][:].rearrange("p (h d) -> p h d", d=HD),
                    )
                nc.vector.memset(vt[:, :, HD:], 1.0)
                v_sb.append(vt)

            # --- bounce K^T and V to DRAM, AllGather within batch group ---
            kv_in = dram.tile([128, KV_COLS], bf, tag="kv_in", name="kv_in")
            kv_out = dram.tile([4 * 128, KV_COLS], bf, tag="kv_out", name="kv_out")
            nc.sync.dma_start(kv_in[:, 0:2048], kT[:])
            for q in range(QT):
                nc.sync.dma_start(
                    kv_in[:, 2048 + q * 1040: 2048 + (q + 1) * 1040], v_sb[q][:]
                )
            nc.gpsimd.collective_compute(
                "AllGather", mybir.AluOpType.bypass,
                ins=[kv_in[:].opt()], outs=[kv_out[:].opt()],
                replica_groups=[[0, 1, 2, 3], [4, 5, 6, 7]],
            )

            # --- Q projection + norm overlap the AllGather flight ---
            pq = proj_qkv(hT, wq_d, l)
            qT = qknorm_transpose(pq, qn_sb, 0, "qT")

            # prefetch Wo weights during the AllGather/attention
            wo_wt = []
            for ch in range(2):
                wt = p_w.tile([128, NKT, 512], bf, tag="w", name="w")
                nc.sync.dma_start(
                    wt[:], wo_d[l, :, ch * 512:(ch + 1) * 512]
                    .rearrange("(a p) c -> p a c", p=128)
                )
                wo_wt.append(wt)

            kTf = p_slab.tile([128, 4 * NKT, TLOC], bf, tag="kTf", name="kTf")
            vf = p_slab.tile([128, 8, H, HD + 1], bf, tag="vf", name="vf")
            for s in range(4):
                nc.sync.dma_start(
                    kTf[:, s * 8:(s + 1) * 8, :],
                    kv_out[s * 128:(s + 1) * 128, 0:2048],
                )
                nc.sync.dma_start(
                    vf[:, 2 * s:2 * s + 2, :, :],
                    kv_out[s * 128:(s + 1) * 128, 2048:KV_COLS],
                )

            # --- attention: head pairs, dense QK via row-tiled concurrency,
            # software-pipelined so AV(pair d-1) hides exp(pair d) latency ---
            o2 = [p_o2.tile([128, H, HD], f32, tag=f"o2{q}", name=f"o2{q}") for q in range(QT)]

            def qk_exp_pair(d):
                """QK matmuls + exp + mask for head pair (2d, 2d+1)."""
                pe_sb = {}
                for half in range(2):     # key tiles 4*half .. 4*half+3
                    regs = {}
                    for hp in range(2):   # hp=0 -> head 2d (parts 0:64), hp=1 -> 2d+1
                        regs[hp] = ps_big.tile([128, 1024], f32, tag="qk", name="qk")
                    for tt in range(4):
                        t = half * 4 + tt
                        for hp in range(2):
                            lk = kTf[64 * hp: 64 * hp + 64,
                                     (t // 2) * 8 + d,
                                     (t % 2) * 128: (t % 2) * 128 + 128]
                            qr = qT[64 * hp: 64 * hp + 64, d, :]
                            nc.tensor.matmul(
                                regs[hp][:, tt * 256:(tt + 1) * 256], lk, qr,
                                start=True, stop=True,
                            )
                    for hp in range(2):
                        if half == 0:
                            pe_sb[hp] = p_pe.tile([128, 8, TLOC], bf, tag="pe", name="pe")
                        nc.scalar.activation(
                            pe_sb[hp][:, half * 4:(half + 1) * 4, :], regs[hp][:],
                            AF.Exp,
                        )
                        nc.vector.tensor_mul(
                            pe_sb[hp][:, half * 4:(half + 1) * 4, :],
                            pe_sb[hp][:, half * 4:(half + 1) * 4, :],
                            mask_sb[:, half * 4:(half + 1) * 4, :],
                        )
                return pe_sb

            def av_pair(d, pe_sb):
                """AV matmuls + epilogue for head pair (2d, 2d+1)."""
                for hp in range(2):
                    h = 2 * d + hp
                    po = ps.tile([128, 512], f32, tag="ps", name="ps")
                    for t in range(8):
                        nc.tensor.matmul(
                            po[:HD + 1, :TLOC], vf[:, t, h, :], pe_sb[hp][:, t, :],
                            start=(t == 0), stop=(t == 7),
                        )
                    # epilogue: transpose, divide by sums, gate
                    ot = p_h.tile([HD + 1, TLOC], bf, tag="ot", name="ot")
                    nc.vector.tensor_copy(ot[:], po[:HD + 1, :TLOC])
                    for q in range(QT):
                        px = ps.tile([128, 512], bf, tag="ps", name="ps")
                        nc.tensor.transpose(
                            px[:, :HD + 1], ot[:, q * 128:(q + 1) * 128],
                            id_sb[:HD + 1, :HD + 1],
                        )
                        inv = p_small.tile([128, 1], f32, tag="ainv", name="ainv")
                        nc.vector.reciprocal(inv[:], px[:, HD:HD + 1])
                        ivg = p_small.tile([128, 1], f32, tag="aivg", name="aivg")
                        nc.vector.tensor_mul(ivg[:], inv[:], g_sb[:, h:h + 1])
                        nc.vector.tensor_scalar_mul(o2[q][:, h, :], px[:, :HD], ivg[:])

            prev = None
            for d in range(NKT):
                pe_sb = qk_exp_pair(d)
                if prev is not None:
                    av_pair(prev[0], prev[1])
                prev = (d, pe_sb)
            av_pair(prev[0], prev[1])

            # value residual + transpose for Wo
            o2T = p_tr.tile([128, NKT, TLOC], bf, tag="o2T", name="o2T")
            for q in range(QT):
                nc.vector.tensor_add(o2[q][:], o2[q][:], v_sb[q][:, :, :HD])
                o2b = p_h.tile([128, D], bf, tag="sc", name="o2b")
                nc.vector.tensor_copy(o2b[:], o2[q][:].rearrange("p h d -> p (h d)"))
                for d in range(NKT):
                    pt = ps.tile([128, 512], bf, tag="ps", name="ps")
                    nc.tensor.transpose(pt[:, :128], o2b[:, d * 128:(d + 1) * 128], id_sb[:])
                    nc.vector.tensor_copy(o2T[:, d, q * 128:(q + 1) * 128], pt[:, :128])

            # --- Wo: x += o2 @ Wo ---
            for ch in range(2):
                pts = [ps.tile([128, 512], f32, tag="ps", name="ps") for _ in range(QT)]
                wt = wo_wt[ch]
                for k in range(NKT):
                    for q in range(QT):
                        nc.tensor.matmul(
                            pts[q][:], o2T[:, k, q * 128:(q + 1) * 128], wt[:, k, :],
                            start=(k == 0), stop=(k == NKT - 1),
                        )
                for q in range(QT):
                    nc.vector.tensor_add(
                        x[q][:, ch * 512:(ch + 1) * 512],
                        x[q][:, ch * 512:(ch + 1) * 512], pts[q][:],
                    )

            # --- MLP ---
            hmT = rms_to_hT("hmT")
            uT = p_uT.tile([128, NFT, TLOC], bf, tag="uT", name="uT")
            for fg in range(NFT // 4):
                w1t = p_w.tile([128, NKT, 4, 128], bf, tag="w", name="w1")
                nc.sync.dma_start(
                    w1t[:],
                    w1_d[l, :, fg * 512:(fg + 1) * 512]
                    .rearrange("(a p) (b f) -> p a b f", p=128, f=128),
                )
                for fi in range(4):
                    fc = fg * 4 + fi
                    pu = ps.tile([128, 512], f32, tag="ps", name="ps")
                    for k in range(NKT):
                        nc.tensor.matmul(
                            pu[:, :TLOC], w1t[:, k, fi, :], hmT[:, k, :],
                            start=(k == 0), stop=(k == NKT - 1),
                        )
                    nc.scalar.activation(uT[:, fc, :], pu[:, :TLOC], AF.Gelu)
            for ch in range(2):
                pts = [ps.tile([128, 512], f32, tag="ps", name="ps") for _ in range(QT)]
                for fgg in range(4):
                    w2t = p_w.tile([128, NKT, 512], bf, tag="w", name="w")
                    nc.sync.dma_start(
                        w2t[:], w2_d[l, fgg * 1024:(fgg + 1) * 1024,
                                     ch * 512:(ch + 1) * 512]
                        .rearrange("(a p) c -> p a c", p=128)
                    )
                    for ki in range(NKT):
                        fc = fgg * 8 + ki
                        for q in range(QT):
                            nc.tensor.matmul(
                                pts[q][:], uT[:, fc, q * 128:(q + 1) * 128],
                                w2t[:, ki, :],
                                start=(fc == 0), stop=(fc == NFT - 1),
                            )
                for q in range(QT):
                    nc.vector.tensor_add(
                        x[q][:, ch * 512:(ch + 1) * 512],
                        x[q][:, ch * 512:(ch + 1) * 512], pts[q][:],
                    )

        # ---------------- final norm + LM head ----------------
        hfT = rms_to_hT("hT")
        lm_in = dram.tile([128, 2048], bf, tag="lm_in", name="lm_in")
        lm_out = dram.tile([8 * 128, 2048], bf, tag="lm_out", name="lm_out", addr_space="Shared")
        nc.sync.dma_start(lm_in[:, :], hfT[:])
        nc.gpsimd.collective_compute(
            "AllGather", mybir.AluOpType.bypass,
            ins=[lm_in[:].opt()], outs=[lm_out[:].opt()],
            replica_groups=[[0, 1, 2, 3, 4, 5, 6, 7]],
        )
        hfa = p_lmh.tile([128, 8 * NKT, TLOC], bf, tag="hfT", name="hfT")
        for s in range(8):
            nc.sync.dma_start(
                hfa[:, s * 8:(s + 1) * 8, :],
                lm_out[s * 128:(s + 1) * 128, :],
            )
        for vt in range(NVC):
            wl = p_w.tile([128, NKT, 512], bf, tag="w", name="wlm")
            nc.sync.dma_start(
                wl[:],
                wlm_d[:, vt * 512:(vt + 1) * 512]
                .rearrange("(a p) c -> p a c", p=128),
            )
            for tg in range(4):
                ob = p_out.tile([128, 4, 512], bf, tag="ob", name="ob")
                for ti in range(4):
                    tt = tg * 4 + ti
                    pl = ps.tile([128, 512], f32, tag="ps", name="ps")
                    for k in range(NKT):
                        nc.tensor.matmul(
                            pl[:],
                            hfa[:, (tt // 2) * 8 + k,
                                (tt % 2) * 128:(tt % 2) * 128 + 128],
                            wl[:, k, :],
                            start=(k == 0), stop=(k == NKT - 1),
                        )
                    nc.scalar.activation(ob[:, ti, :], pl[:], AF.Copy)
                nc.sync.dma_start(
                    out_d[tg * 512:(tg + 1) * 512, vt * 512:(vt + 1) * 512]
                    .rearrange("(b p) c -> p b c", p=128),
                    ob[:],
                )
    nc.compile()
    return nc


# ---------------------------------------------------------------------------
# host side
# ---------------------------------------------------------------------------

def _prep_inputs(inputs, n_layers=L_ALL):
    ids = np.asarray(inputs["input_ids"])
    embed = np.asarray(inputs["embed"], np.float32)
    pos = np.asarray(inputs["pos_embed"], np.float32)
    ln1 = np.asarray(inputs["ln1_w"], np.float32)
    ln2 = np.asarray(inputs["ln2_w"], np.float32)
    qn = np.asarray(inputs["qn_w"], np.float32)
    kn = np.asarray(inputs["kn_w"], np.float32)
    gate = np.asarray(inputs["gate"], np.float32)
    lnf = np.asarray(inputs["lnf_w"], np.float32)

    wq = (ln1[:, :, None] * np.asarray(inputs["Wq"], np.float32)).astype(BF16)
    wk = (ln1[:, :, None] * np.asarray(inputs["Wk"], np.float32)).astype(BF16)
    wv = (ln1[:, :, None] * np.asarray(inputs["Wv"], np.float32)).astype(BF16)
    wo = np.asarray(inputs["Wo"], np.float32).astype(BF16)
    w1 = (ln2[:, :, None] * np.asarray(inputs["W1"], np.float32)).astype(BF16)
    w2 = np.asarray(inputs["W2"], np.float32).astype(BF16)
    wlm_full = lnf[:, None] * np.asarray(inputs["Wlm"], np.float32)
    wlm_pad = np.zeros((D, VS * NCORE), np.float32)
    wlm_pad[:, :V] = wlm_full
    wlm_pad = wlm_pad.astype(BF16)

    # qn/kn replicated [L, 2, 128, D]; 1/sqrt(HD) folded into kn side
    qnk = np.zeros((n_layers, 2, 128, D), np.float32)
    for l in range(n_layers):
        qnk[l, 0, :, :] = np.tile(qn[l], H)[None, :]
        qnk[l, 1, :, :] = np.tile(kn[l] / np.sqrt(HD), H)[None, :]
    qnk = qnk.astype(BF16)
    g_rep = np.broadcast_to(gate[:n_layers, None, :], (n_layers, 128, H)).copy()

    in_maps = []
    for c in range(NCORE):
        b, a = divmod(c, 4)
        t0 = a * TLOC
        toks = ids[b, t0:t0 + TLOC].astype(np.int64)
        x0 = embed[toks] + pos[t0:t0 + TLOC]
        # causal mask [8 key tiles, 128 key pos, 256 query pos]
        kg = np.arange(T).reshape(8, 128)
        qg = t0 + np.arange(TLOC)
        mask = (kg[:, :, None] <= qg[None, None, :]).astype(BF16)
        in_maps.append({
            "x0": x0.astype(np.float32),
            "maskt": mask,
            "wq": wq[:n_layers], "wk": wk[:n_layers], "wv": wv[:n_layers],
            "wo": wo[:n_layers], "w1": w1[:n_layers], "w2": w2[:n_layers],
            "qnk": qnk, "g": g_rep.astype(np.float32),
            "wlm": wlm_pad[:, c * VS:(c + 1) * VS],
        })
    return in_maps


_NC_CACHE = {}


def _get_nc(n_layers=L_ALL):
    if n_layers not in _NC_CACHE:
        _NC_CACHE[n_layers] = build_nc(n_layers)
    return _NC_CACHE[n_layers]


def _install_profile_hook():
    """Recreate antenv.axon_hooks with an NTFF profile hook via ctypes."""
    import sys as _sys, types, ctypes, contextlib, os
    try:
        import antenv.axon_hooks  # noqa: F401
        return
    except ImportError:
        pass
    so_path = os.environ.get("PJRT_LIBRARY_PATH", "/opt/axon/libaxon_pjrt.so")
    lib = ctypes.CDLL(so_path)
    if not hasattr(lib, "axon_start_nrt_profile"):
        return
    lib.axon_start_nrt_profile.argtypes = [ctypes.POINTER(ctypes.c_int64), ctypes.c_size_t]
    lib.axon_start_nrt_profile.restype = ctypes.c_int64
    lib.axon_stop_nrt_profile.argtypes = [ctypes.c_char_p]
    lib.axon_stop_nrt_profile.restype = ctypes.c_int64

    @contextlib.contextmanager
    def _hook(output_dir, device_ids):
        import jax
        jax.devices()
        if device_ids:
            ids = (ctypes.c_int64 * len(device_ids))(*device_ids)
            rc = lib.axon_start_nrt_profile(ids, len(device_ids))
        else:
            rc = lib.axon_start_nrt_profile(None, 0)
        if rc != 0:
            raise RuntimeError(f"axon_start_nrt_profile rc={rc}")
        try:
            yield
        finally:
            n = lib.axon_stop_nrt_profile(str(output_dir).encode())
            print(f"profile: {n} file(s) written to {output_dir}")

    import antenv
    mod = types.ModuleType("antenv.axon_hooks")
    _state = {"hook": _hook}
    mod.set_axon_ntff_profile_hook = lambda h: _state.__setitem__("hook", h)
    mod.get_axon_ntff_profile_hook = lambda: _state["hook"]
    _sys.modules["antenv.axon_hooks"] = mod
    antenv.axon_hooks = mod


def run(inputs, n_layers=L_ALL, trace=False):
    from concourse.bass_utils import run_bass_kernel_spmd
    if trace:
        _install_profile_hook()
    nc = _get_nc(n_layers)
    in_maps = _prep_inputs(inputs, n_layers)
    res = run_bass_kernel_spmd(
        nc, in_maps, core_ids=list(range(NCORE)), trace=trace,
    )
    outs = [np.asarray(r["out"], dtype=np.float32) for r in res.results]
    logits = np.concatenate(outs, axis=1)[:, :V]
    return logits.reshape(B, T, V), res


def kernel(**inputs):
    logits, _ = run(inputs)
    return logits


# revision 14
# speedup vs baseline: 1.2082x; 1.0934x over previous
"""Trainium2 Bass kernel for a 12-layer GPT LM (CodeGPTLMHeadModel).

Sharding (8 NeuronCores, one chip):
  - Layer stack: tokens sharded. B=2 batches x 1024 tokens; cores 0-3 own
    batch 0, cores 4-7 batch 1; core c owns 256 contiguous tokens
    (chunks 2a, 2a+1 with a = c%4).  All weights replicated, streamed
    from HBM.  Attention: each core computes q/k/v for its local tokens,
    AllGathers K^T/V inside its 4-core batch group, then computes all 16
    heads for its local 256 queries (causal handled by a per-core mask
    input => uniform SPMD graph).
  - LM head: vocab sharded.  AllGather of the final hidden states across
    all 8 cores; each core computes a 6656-wide padded vocab slice.
  - Norm weights (ln1/ln2/lnf) are folded into the following matmul
    weights host-side; qn/kn/gate are applied on-device from replicated
    constant inputs.  Matmuls run in bf16 (f32 residual/psum).

v2 perf notes:
  - rsqrt via Ln+Exp so rms/qknorm/attention all live in the
    natural_log_exp activation table set (only Gelu switches sets).
  - attention: per head-pair, QK matmuls for 4 key tiles are issued
    interleaved (partitions 0-63 / 64-127 -> concurrent row-tiled MMs),
    exp runs on [128,1024] psum regions (1 ACT inst per 4 key tiles),
    mask is one tensor_tensor, AV matmuls run back-to-back.
  - K/V computed first, AllGather launched, all Q-side work overlaps it.
"""

import numpy as np
import ml_dtypes

BF16 = ml_dtypes.bfloat16

L_ALL, B, T, D, H, HD, F, V = 12, 2, 1024, 1024, 16, 64, 4096, 50257
NCORE = 8
TLOC = 256            # tokens per core
QT = TLOC // 128      # 2 token tiles of 128
NKT = D // 128        # 8 contraction tiles over D
NFT = F // 128        # 32 tiles over F
VS = 6656             # padded vocab shard per core (13 * 512)
NVC = VS // 512       # 13 vocab chunks of 512
EPS = 1e-5

# kv bounce layout (bf16): one row per partition: k^T 2048 cols + v 2*1040 cols
KV_COLS = 2048 + 2 * 1040  # 4128


def build_nc(n_layers=L_ALL):
    from contextlib import ExitStack
    from concourse import bass, bacc, mybir, tile

    f32 = mybir.dt.float32
    bf = mybir.dt.bfloat16
    AF = mybir.ActivationFunctionType

    # Patch the activation-table list used by the set-selection pass so Exp
    # and Ln first-match to natural_log_exp_and_others (which really contains
    # both).  Indices/order are preserved, only membership used for selection
    # is edited, so the emitted act_func_set_id still points at the right
    # runtime tables.  Without this the Ln->Exp rsqrt chain ping-pongs between
    # the ln-only and exp-only sets, costing two ~1.3us table loads per rms.
    import concourse.bacc as _bacc_mod
    from concourse.hw_specs import get_activation_tables as _orig_tabs
    if getattr(_bacc_mod, "_act_tabs_patched", None) is None:
        def _patched_tabs(arch, _orig=_orig_tabs):
            out = {}
            for name, fns in _orig(arch).items():
                fns = set(fns)
                if name != "natural_log_exp_and_others":
                    fns.discard(AF.Exp)
                    fns.discard(AF.Ln)
                out[name] = fns
            return out
        _bacc_mod.get_activation_tables = _patched_tabs
        _bacc_mod._act_tabs_patched = True

    nc = bacc.Bacc(None, target_bir_lowering=False, debug=False)

    # ---------------- external parameters (per-core shards) ----------------
    x0_d = nc.dram_tensor("x0", [TLOC, D], f32, kind="ExternalInput")
    maskt = nc.dram_tensor("maskt", [8, 128, TLOC], bf, kind="ExternalInput")
    wq_d = nc.dram_tensor("wq", [n_layers, D, D], bf, kind="ExternalInput")
    wk_d = nc.dram_tensor("wk", [n_layers, D, D], bf, kind="ExternalInput")
    wv_d = nc.dram_tensor("wv", [n_layers, D, D], bf, kind="ExternalInput")
    wo_d = nc.dram_tensor("wo", [n_layers, D, D], bf, kind="ExternalInput")
    w1_d = nc.dram_tensor("w1", [n_layers, D, F], bf, kind="ExternalInput")
    w2_d = nc.dram_tensor("w2", [n_layers, F, D], bf, kind="ExternalInput")
    qnk_d = nc.dram_tensor("qnk", [n_layers, 2, 128, D], bf, kind="ExternalInput")
    g_d = nc.dram_tensor("g", [n_layers, 128, H], f32, kind="ExternalInput")
    wlm_d = nc.dram_tensor("wlm", [D, VS], bf, kind="ExternalInput")
    out_d = nc.dram_tensor("out", [B * T, VS], bf, kind="ExternalOutput")

    id_np = np.eye(128, dtype=BF16)
    id_dram = nc.inline_tensor(id_np, name="id128")

    with tile.TileContext(nc) as tc, ExitStack() as ctx:
        ep = ctx.enter_context

        consts = ep(tc.tile_pool(name="consts", bufs=1))
        p_res = ep(tc.tile_pool(name="p_res", bufs=1))
        p_h = ep(tc.tile_pool(name="p_h", bufs=2))
        p_tr = ep(tc.tile_pool(name="p_tr", bufs=1))
        p_small = ep(tc.tile_pool(name="p_small", bufs=8))
        p_v = ep(tc.tile_pool(name="p_v", bufs=1))
        p_o2 = ep(tc.tile_pool(name="p_o2", bufs=1))
        p_uT = ep(tc.tile_pool(name="p_uT", bufs=1))
        p_slab = ep(tc.tile_pool(name="p_slab", bufs=1))
        p_w = ep(tc.tile_pool(name="p_w", bufs=4))
        p_lmh = ep(tc.tile_pool(name="p_lmh", bufs=1))
        p_out = ep(tc.tile_pool(name="p_out", bufs=2))
        p_qn = ep(tc.tile_pool(name="p_qn", bufs=1))
        p_pe = ep(tc.tile_pool(name="p_pe", bufs=4))
        # PSUM: 8 banks total.  ps_big = 2 x [128,1024] f32 (2 banks each),
        # ps = 4 x [128,512] f32 (1 bank each).
        ps_big = ep(tc.tile_pool(name="ps_big", bufs=2, space="PSUM"))
        ps = ep(tc.tile_pool(name="ps", bufs=4, space="PSUM"))
        dram = ep(tc.tile_pool(name="dram", bufs=2, space="DRAM"))

        # ---------------- constants into SBUF ----------------
        id_sb = consts.tile([128, 128], bf, tag="id", name="id")
        nc.sync.dma_start(id_sb[:], id_dram[:, :])
        mask_sb = consts.tile([128, 8, TLOC], bf, tag="mask", name="mask")
        nc.sync.dma_start(mask_sb[:], maskt[:, :, :].rearrange("t p q -> p t q"))
        eps_sb = consts.tile([128, 1], f32, tag="eps", name="eps")
        nc.vector.memset(eps_sb[:], EPS)

        # ---------------- embedding ----------------
        x = [p_res.tile([128, D], f32, tag=f"x{q}", name=f"x{q}") for q in range(QT)]

        for q in range(QT):
            nc.sync.dma_start(x[q][:], x0_d[q * 128:(q + 1) * 128, :])

        # per-head Q slots: head 2d lives in partitions 0:64 of slot 2d,
        # head 2d+1 in partitions 64:128 of slot 2d+1; the other half of
        # each slot stays zero forever so QK matmuls can contract over the
        # full 128 partitions (full-array MMs keep the PE clock warm).
        qTz = p_tr.tile([128, H, TLOC], bf, tag="qTz", name="qTz")
        nc.vector.memset(qTz[:], 0.0)

        # ---------------- helpers ----------------
        def rsqrt_act(dst, src, scale):
            """dst = rsqrt(src*scale + eps) using Ln+Exp (stays in the
            natural_log_exp table set alongside attention's Exp)."""
            lt = p_small.tile(list(dst.shape), f32, tag="lnt", name="lnt")
            nc.scalar.activation(lt[:], src, AF.Ln, scale=scale, bias=eps_sb[:])
            nc.scalar.activation(dst, lt[:], AF.Exp, scale=-0.5)

        def rms_to_hT(tag):
            """RMS-normalize x (token-major) -> h bf16 -> transposed hT[128,8,256].

            The two Square halves are separate ACT instructions so the ch0
            square can overlap the producer's ch1 matmuls."""
            hT = p_tr.tile([128, NKT, TLOC], bf, tag=tag)
            for q in range(QT):
                sq = p_h.tile([128, D], bf, tag="sc", name="sq")
                sss = p_small.tile([128, 2], f32, tag="ssq", name="ssq")
                for ch in range(2):
                    nc.scalar.activation(
                        sq[:, ch * 512:(ch + 1) * 512],
                        x[q][:, ch * 512:(ch + 1) * 512],
                        AF.Square, accum_out=sss[:, ch:ch + 1],
                    )
                ssq = p_small.tile([128, 1], f32, tag="ssqt", name="ssqt")
                nc.vector.tensor_add(ssq[:], sss[:, 0:1], sss[:, 1:2])
                inv = p_small.tile([128, 1], f32, tag="inv", name="inv")
                rsqrt_act(inv[:], ssq[:], 1.0 / D)
                h = p_h.tile([128, D], bf, tag="h", name="h")
                nc.vector.tensor_scalar_mul(h[:], x[q][:], inv[:])
                for d in range(NKT):
                    pt = ps.tile([128, 512], bf, tag="ps", name="ps")
                    nc.tensor.transpose(pt[:, :128], h[:, d * 128:(d + 1) * 128], id_sb[:])
                    nc.vector.tensor_copy(hT[:, d, q * 128:(q + 1) * 128], pt[:, :128])
            return hT

        def proj_qkv(hT, w_dram, l):
            """form-1 projection: out[128tok, D] psum pair per (q, ch)."""
            outs = {}
            for ch in range(2):
                pts = [ps.tile([128, 512], f32, tag="ps", name="ps") for _ in range(QT)]
                wt = p_w.tile([128, NKT, 512], bf, tag="w", name="w")
                nc.sync.dma_start(
                    wt[:], w_dram[l, :, ch * 512:(ch + 1) * 512]
                    .rearrange("(a p) c -> p a c", p=128)
                )
                for k in range(NKT):
                    for q in range(QT):
                        nc.tensor.matmul(
                            pts[q][:], hT[:, k, q * 128:(q + 1) * 128], wt[:, k, :],
                            start=(k == 0), stop=(k == NKT - 1),
                        )
                outs[ch] = pts
            return outs

        def qknorm_transpose(pq, qn_sb, which, tag, zslots=None):
            """QK-norm (token-major, from psum) + qn/kn apply + transpose.

            pq: dict ch -> [QT] psum tiles [128, 512] (= [128, 8, 64])
            returns qT [128, 8, 256] bf16 (partition = dim%128 within pairs of
            heads), or writes into zslots [128, H, 256] (per-head slots with
            the other head's partition half left zero, for full-K QK matmuls).
            """
            qT = zslots if zslots is not None else p_tr.tile([128, NKT, TLOC], bf, tag=tag)
            for q in range(QT):
                ss = p_small.tile([128, H], f32, tag="ssqk", name="ssqk")
                for ch in range(2):
                    sqs = p_h.tile([128, 512], bf, tag="sc", name="sqs")
                    nc.scalar.activation(sqs[:], pq[ch][q][:], AF.Square)
                    nc.vector.tensor_reduce(
                        ss[:, ch * 8:(ch + 1) * 8],
                        sqs[:].rearrange("p (h d) -> p h d", d=HD),
                        axis=mybir.AxisListType.X, op=mybir.AluOpType.add,
                    )
                iv = p_small.tile([128, H], f32, tag="ivqk", name="ivqk")
                rsqrt_act(iv[:], ss[:], 1.0 / HD)
                qh = p_h.tile([128, D], bf, tag="qh", name="qh")
                for ch in range(2):
                    tmp = p_h.tile([128, 512], f32, tag="sc", name="qtmp")
                    nc.vector.tensor_tensor(
                        tmp[:].rearrange("p (h d) -> p h d", d=HD),
                        pq[ch][q][:].rearrange("p (h d) -> p h d", d=HD),
                        iv[:, ch * 8:(ch + 1) * 8, None].to_broadcast((128, 8, HD)),
                        op=mybir.AluOpType.mult,
                    )
                    nc.vector.tensor_mul(
                        qh[:, ch * 512:(ch + 1) * 512], tmp[:],
                        qn_sb[:, which, ch * 512:(ch + 1) * 512],
                    )
                for d in range(NKT):
                    pt = ps.tile([128, 512], bf, tag="ps", name="ps")
                    nc.tensor.transpose(pt[:, :128], qh[:, d * 128:(d + 1) * 128], id_sb[:])
                    if zslots is not None:
                        nc.vector.tensor_copy(
                            qT[0:64, 2 * d, q * 128:(q + 1) * 128], pt[0:64, :128])
                        nc.vector.tensor_copy(
                            qT[64:128, 2 * d + 1, q * 128:(q + 1) * 128], pt[64:128, :128])
                    else:
                        nc.vector.tensor_copy(qT[:, d, q * 128:(q + 1) * 128], pt[:, :128])
            return qT

        # ---------------- layers ----------------
        for l in range(n_layers):
            qn_sb = p_qn.tile([128, 2, D], bf, tag="qn", name="qn")
            nc.sync.dma_start(qn_sb[:], qnk_d[l, :, :, :].rearrange("a p d -> p a d"))
            g_sb = p_qn.tile([128, H], f32, tag="g", name="g")
            nc.sync.dma_start(g_sb[:], g_d[l, :, :])

            hT = rms_to_hT("hT")

            # --- K and V first, so the AllGather can launch early ---
            pk = proj_qkv(hT, wk_d, l)
            kT = qknorm_transpose(pk, qn_sb, 1, "kT")

            pv = proj_qkv(hT, wv_d, l)
            # v eviction: [128, 16, 65] with ones in col 64
            v_sb = []
            for q in range(QT):
                vt = p_v.tile([128, H, HD + 1], bf, tag=f"v{q}", name=f"v{q}")
                for ch in range(2):
                    nc.vector.tensor_copy(
                        vt[:, ch * 8:(ch + 1) * 8, :HD],
                        pv[ch][q][:].rearrange("p (h d) -> p h d", d=HD),
                    )
                nc.vector.memset(vt[:, :, HD:], 1.0)
                v_sb.append(vt)

            # --- bounce K^T and V to DRAM, AllGather within batch group ---
            kv_in = dram.tile([128, KV_COLS], bf, tag="kv_in", name="kv_in")
            kv_out = dram.tile([4 * 128, KV_COLS], bf, tag="kv_out", name="kv_out",
                               addr_space="Shared")
            nc.sync.dma_start(kv_in[:, 0:2048], kT[:])
            for q in range(QT):
                nc.sync.dma_start(
                    kv_in[:, 2048 + q * 1040: 2048 + (q + 1) * 1040], v_sb[q][:]
                )
            nc.gpsimd.collective_compute(
                "AllGather", mybir.AluOpType.bypass,
                ins=[kv_in[:].opt()], outs=[kv_out[:].opt()],
                replica_groups=[[0, 1, 2, 3], [4, 5, 6, 7]],
            )

            # --- Q projection + norm overlap the AllGather flight ---
            pq = proj_qkv(hT, wq_d, l)
            qknorm_transpose(pq, qn_sb, 0, "qT", zslots=qTz)

            # prefetch Wo weights during the AllGather/attention
            wo_wt = []
            for ch in range(2):
                wt = p_w.tile([128, NKT, 512], bf, tag="w", name="w")
                nc.sync.dma_start(
                    wt[:], wo_d[l, :, ch * 512:(ch + 1) * 512]
                    .rearrange("(a p) c -> p a c", p=128)
                )
                wo_wt.append(wt)

            kTf = p_slab.tile([128, 4 * NKT, TLOC], bf, tag="kTf", name="kTf")
            vf = p_slab.tile([128, 8, H, HD + 1], bf, tag="vf", name="vf")
            for s in range(4):
                nc.sync.dma_start(
                    kTf[:, s * 8:(s + 1) * 8, :],
                    kv_out[s * 128:(s + 1) * 128, 0:2048],
                )
                nc.sync.dma_start(
                    vf[:, 2 * s:2 * s + 2, :, :],
                    kv_out[s * 128:(s + 1) * 128, 2048:KV_COLS],
                )

            # --- attention: head pairs, dense QK via row-tiled concurrency,
            # software-pipelined so AV(pair d-1) hides exp(pair d) latency ---
            o2 = [p_o2.tile([128, H, HD], f32, tag=f"o2{q}", name=f"o2{q}") for q in range(QT)]

            def qk_exp_pair(d):
                """QK matmuls + exp + mask for head pair (2d, 2d+1).

                lhsT is the full 128-partition K tile; the per-head Q slot has
                the other head's partitions zeroed, so each matmul is a
                full-array 128x128x256 (counts as PE activity for the HAM
                clock gate, unlike 64-row matmuls)."""
                pe_sb = {}
                for half in range(2):     # key tiles 4*half .. 4*half+3
                    regs = {}
                    for hp in range(2):
                        regs[hp] = ps_big.tile([128, 1024], f32, tag="qk", name="qk")
                    for tt in range(4):
                        t = half * 4 + tt
                        lk = kTf[:, (t // 2) * 8 + d,
                                 (t % 2) * 128: (t % 2) * 128 + 128]
                        for hp in range(2):
                            nc.tensor.matmul(
                                regs[hp][:, tt * 256:(tt + 1) * 256], lk,
                                qTz[:, 2 * d + hp, :],
                                start=True, stop=True,
                            )
                    for hp in range(2):
                        if half == 0:
                            pe_sb[hp] = p_pe.tile([128, 8, TLOC], bf, tag="pe", name="pe")
                        nc.scalar.activation(
                            pe_sb[hp][:, half * 4:(half + 1) * 4, :], regs[hp][:],
                            AF.Exp,
                        )
                        nc.vector.tensor_mul(
                            pe_sb[hp][:, half * 4:(half + 1) * 4, :],
                            pe_sb[hp][:, half * 4:(half + 1) * 4, :],
                            mask_sb[:, half * 4:(half + 1) * 4, :],
                        )
                return pe_sb

            def av_pair(d, pe_sb):
                """AV matmuls + epilogue for head pair (2d, 2d+1)."""
                for hp in range(2):
                    h = 2 * d + hp
                    po = ps.tile([128, 512], f32, tag="ps", name="ps")
                    for t in range(8):
                        nc.tensor.matmul(
                            po[:HD + 1, :TLOC], vf[:, t, h, :], pe_sb[hp][:, t, :],
                            start=(t == 0), stop=(t == 7),
                        )
                    # epilogue: transpose, divide by sums, gate
                    ot = p_h.tile([HD + 1, TLOC], bf, tag="ot", name="ot")
                    nc.vector.tensor_copy(ot[:], po[:HD + 1, :TLOC])
                    for q in range(QT):
                        px = ps.tile([128, 512], bf, tag="ps", name="ps")
                        nc.tensor.transpose(
                            px[:, :HD + 1], ot[:, q * 128:(q + 1) * 128],
                            id_sb[:HD + 1, :HD + 1],
                        )
                        inv = p_small.tile([128, 1], f32, tag="ainv", name="ainv")
                        nc.vector.reciprocal(inv[:], px[:, HD:HD + 1])
                        ivg = p_small.tile([128, 1], f32, tag="aivg", name="aivg")
                        nc.vector.tensor_mul(ivg[:], inv[:], g_sb[:, h:h + 1])
                        nc.vector.tensor_scalar_mul(o2[q][:, h, :], px[:, :HD], ivg[:])

            prev = None
            for d in range(NKT):
                pe_sb = qk_exp_pair(d)
                if prev is not None:
                    av_pair(prev[0], prev[1])
                prev = (d, pe_sb)
            av_pair(prev[0], prev[1])

            # value residual + transpose for Wo
            o2T = p_tr.tile([128, NKT, TLOC], bf, tag="o2T", name="o2T")
            for q in range(QT):
                nc.vector.tensor_add(o2[q][:], o2[q][:], v_sb[q][:, :, :HD])
                o2b = p_h.tile([128, D], bf, tag="sc", name="o2b")
                nc.vector.tensor_copy(o2b[:], o2[q][:].rearrange("p h d -> p (h d)"))
                for d in range(NKT):
                    pt = ps.tile([128, 512], bf, tag="ps", name="ps")
                    nc.tensor.transpose(pt[:, :128], o2b[:, d * 128:(d + 1) * 128], id_sb[:])
                    nc.vector.tensor_copy(o2T[:, d, q * 128:(q + 1) * 128], pt[:, :128])

            # --- Wo: x += o2 @ Wo ---
            for ch in range(2):
                pts = [ps.tile([128, 512], f32, tag="ps", name="ps") for _ in range(QT)]
                wt = wo_wt[ch]
                for k in range(NKT):
                    for q in range(QT):
                        nc.tensor.matmul(
                            pts[q][:], o2T[:, k, q * 128:(q + 1) * 128], wt[:, k, :],
                            start=(k == 0), stop=(k == NKT - 1),
                        )
                for q in range(QT):
                    nc.vector.tensor_add(
                        x[q][:, ch * 512:(ch + 1) * 512],
                        x[q][:, ch * 512:(ch + 1) * 512], pts[q][:],
                    )

            # --- MLP ---
            hmT = rms_to_hT("hmT")
            uT = p_uT.tile([128, NFT, TLOC], bf, tag="uT", name="uT")
            for fg in range(NFT // 4):
                w1t = p_w.tile([128, NKT, 4, 128], bf, tag="w", name="w1")
                nc.sync.dma_start(
                    w1t[:],
                    w1_d[l, :, fg * 512:(fg + 1) * 512]
                    .rearrange("(a p) (b f) -> p a b f", p=128, f=128),
                )
                for fi in range(4):
                    fc = fg * 4 + fi
                    pu = ps.tile([128, 512], f32, tag="ps", name="ps")
                    for k in range(NKT):
                        nc.tensor.matmul(
                            pu[:, :TLOC], w1t[:, k, fi, :], hmT[:, k, :],
                            start=(k == 0), stop=(k == NKT - 1),
                        )
                    nc.scalar.activation(uT[:, fc, :], pu[:, :TLOC], AF.Gelu)
            for ch in range(2):
                pts = [ps.tile([128, 512], f32, tag="ps", name="ps") for _ in range(QT)]
                for fgg in range(4):
                    w2t = p_w.tile([128, NKT, 512], bf, tag="w", name="w")
                    nc.sync.dma_start(
                        w2t[:], w2_d[l, fgg * 1024:(fgg + 1) * 1024,
                                     ch * 512:(ch + 1) * 512]
                        .rearrange("(a p) c -> p a c", p=128)
                    )
                    for ki in range(NKT):
                        fc = fgg * 8 + ki
                        for q in range(QT):
                            nc.tensor.matmul(
                                pts[q][:], uT[:, fc, q * 128:(q + 1) * 128],
                                w2t[:, ki, :],
                                start=(fc == 0), stop=(fc == NFT - 1),
                            )
                for q in range(QT):
                    nc.vector.tensor_add(
                        x[q][:, ch * 512:(ch + 1) * 512],
                        x[q][:, ch * 512:(ch + 1) * 512], pts[q][:],
                    )

        # ---------------- final norm + LM head ----------------
        hfT = rms_to_hT("hT")
        lm_in = dram.tile([128, 2048], bf, tag="lm_in", name="lm_in")
        lm_out = dram.tile([8 * 128, 2048], bf, tag="lm_out", name="lm_out", addr_space="Shared")
        nc.sync.dma_start(lm_in[:, :], hfT[:])
        nc.gpsimd.collective_compute(
            "AllGather", mybir.AluOpType.bypass,
            ins=[lm_in[:].opt()], outs=[lm_out[:].opt()],
            replica_groups=[[0, 1, 2, 3, 4, 5, 6, 7]],
        )
        hfa = p_lmh.tile([128, 8 * NKT, TLOC], bf, tag="hfT", name="hfT")
        for s in range(8):
            nc.sync.dma_start(
                hfa[:, s * 8:(s + 1) * 8, :],
                lm_out[s * 128:(s + 1) * 128, :],
            )
        for vt in range(NVC):
            wl = p_w.tile([128, NKT, 512], bf, tag="w", name="wlm")
            nc.sync.dma_start(
                wl[:],
                wlm_d[:, vt * 512:(vt + 1) * 512]
                .rearrange("(a p) c -> p a c", p=128),
            )
            for tg in range(4):
                ob = p_out.tile([128, 4, 512], bf, tag="ob", name="ob")
                for ti in range(4):
                    tt = tg * 4 + ti
                    pl = ps.tile([128, 512], f32, tag="ps", name="ps")
                    for k in range(NKT):
                        nc.tensor.matmul(
                            pl[:],
                            hfa[:, (tt // 2) * 8 + k,
                                (tt % 2) * 128:(tt % 2) * 128 + 128],
                            wl[:, k, :],
                            start=(k == 0), stop=(k == NKT - 1),
                        )
                    nc.scalar.activation(ob[:, ti, :], pl[:], AF.Copy)
                nc.sync.dma_start(
                    out_d[tg * 512:(tg + 1) * 512, vt * 512:(vt + 1) * 512]
                    .rearrange("(b p) c -> p b c", p=128),
                    ob[:],
                )
    nc.compile()
    return nc


# ---------------------------------------------------------------------------
# host side
# ---------------------------------------------------------------------------

def _prep_inputs(inputs, n_layers=L_ALL):
    ids = np.asarray(inputs["input_ids"])
    embed = np.asarray(inputs["embed"], np.float32)
    pos = np.asarray(inputs["pos_embed"], np.float32)
    ln1 = np.asarray(inputs["ln1_w"], np.float32)
    ln2 = np.asarray(inputs["ln2_w"], np.float32)
    qn = np.asarray(inputs["qn_w"], np.float32)
    kn = np.asarray(inputs["kn_w"], np.float32)
    gate = np.asarray(inputs["gate"], np.float32)
    lnf = np.asarray(inputs["lnf_w"], np.float32)

    wq = (ln1[:, :, None] * np.asarray(inputs["Wq"], np.float32)).astype(BF16)
    wk = (ln1[:, :, None] * np.asarray(inputs["Wk"], np.float32)).astype(BF16)
    wv = (ln1[:, :, None] * np.asarray(inputs["Wv"], np.float32)).astype(BF16)
    wo = np.asarray(inputs["Wo"], np.float32).astype(BF16)
    w1 = (ln2[:, :, None] * np.asarray(inputs["W1"], np.float32)).astype(BF16)
    w2 = np.asarray(inputs["W2"], np.float32).astype(BF16)
    wlm_full = lnf[:, None] * np.asarray(inputs["Wlm"], np.float32)
    wlm_pad = np.zeros((D, VS * NCORE), np.float32)
    wlm_pad[:, :V] = wlm_full
    wlm_pad = wlm_pad.astype(BF16)

    # qn/kn replicated [L, 2, 128, D]; 1/sqrt(HD) folded into kn side
    qnk = np.zeros((n_layers, 2, 128, D), np.float32)
    for l in range(n_layers):
        qnk[l, 0, :, :] = np.tile(qn[l], H)[None, :]
        qnk[l, 1, :, :] = np.tile(kn[l] / np.sqrt(HD), H)[None, :]
    qnk = qnk.astype(BF16)
    g_rep = np.broadcast_to(gate[:n_layers, None, :], (n_layers, 128, H)).copy()

    in_maps = []
    for c in range(NCORE):
        b, a = divmod(c, 4)
        t0 = a * TLOC
        toks = ids[b, t0:t0 + TLOC].astype(np.int64)
        x0 = embed[toks] + pos[t0:t0 + TLOC]
        # causal mask [8 key tiles, 128 key pos, 256 query pos]
        kg = np.arange(T).reshape(8, 128)
        qg = t0 + np.arange(TLOC)
        mask = (kg[:, :, None] <= qg[None, None, :]).astype(BF16)
        in_maps.append({
            "x0": x0.astype(np.float32),
            "maskt": mask,
            "wq": wq[:n_layers], "wk": wk[:n_layers], "wv": wv[:n_layers],
            "wo": wo[:n_layers], "w1": w1[:n_layers], "w2": w2[:n_layers],
            "qnk": qnk, "g": g_rep.astype(np.float32),
            "wlm": wlm_pad[:, c * VS:(c + 1) * VS],
        })
    return in_maps


_NC_CACHE = {}


def _get_nc(n_layers=L_ALL):
    if n_layers not in _NC_CACHE:
        _NC_CACHE[n_layers] = build_nc(n_layers)
    return _NC_CACHE[n_layers]


def _install_profile_hook():
    """Recreate antenv.axon_hooks with an NTFF profile hook via ctypes."""
    import sys as _sys, types, ctypes, contextlib, os
    try:
        import antenv.axon_hooks  # noqa: F401
        return
    except ImportError:
        pass
    so_path = os.environ.get("PJRT_LIBRARY_PATH", "/opt/axon/libaxon_pjrt.so")
    lib = ctypes.CDLL(so_path)
    if not hasattr(lib, "axon_start_nrt_profile"):
        return
    lib.axon_start_nrt_profile.argtypes = [ctypes.POINTER(ctypes.c_int64), ctypes.c_size_t]
    lib.axon_start_nrt_profile.restype = ctypes.c_int64
    lib.axon_stop_nrt_profile.argtypes = [ctypes.c_char_p]
    lib.axon_stop_nrt_profile.restype = ctypes.c_int64

    @contextlib.contextmanager
    def _hook(output_dir, device_ids):
        import jax
        jax.devices()
        if device_ids:
            ids = (ctypes.c_int64 * len(device_ids))(*device_ids)
            rc = lib.axon_start_nrt_profile(ids, len(device_ids))
        else:
            rc = lib.axon_start_nrt_profile(None, 0)
        if rc != 0:
            raise RuntimeError(f"axon_start_nrt_profile rc={rc}")
        try:
            yield
        finally:
            n = lib.axon_stop_nrt_profile(str(output_dir).encode())
            print(f"profile: {n} file(s) written to {output_dir}")

    import antenv
    mod = types.ModuleType("antenv.axon_hooks")
    _state = {"hook": _hook}
    mod.set_axon_ntff_profile_hook = lambda h: _state.__setitem__("hook", h)
    mod.get_axon_ntff_profile_hook = lambda: _state["hook"]
    _sys.modules["antenv.axon_hooks"] = mod
    antenv.axon_hooks = mod


def run(inputs, n_layers=L_ALL, trace=False):
    from concourse.bass_utils import run_bass_kernel_spmd
    if trace:
        _install_profile_hook()
    nc = _get_nc(n_layers)
    in_maps = _prep_inputs(inputs, n_layers)
    res = run_bass_kernel_spmd(
        nc, in_maps, core_ids=list(range(NCORE)), trace=trace,
    )
    outs = [np.asarray(r["out"], dtype=np.float32) for r in res.results]
    logits = np.concatenate(outs, axis=1)[:, :V]
    return logits.reshape(B, T, V), res


def kernel(**inputs):
    logits, _ = run(inputs)
    return logits
